# revision 23
# baseline (speedup 1.0000x reference)
"""Trainium2 Bass kernel for batched 8-head local-window attention.

Shapes (hardcoded): x [32, 512, 512], w_qkv [512, 1536], w_proj [512, 512],
b_proj [512], mask [1, 1, 512, 512] additive (0 or -1e30).

Strategy: data-parallel over batch across 8 cores (4 batch elements each).
All matmuls in bf16 (fp32 PSUM accumulation). Layouts chosen so that no
on-device data movement is wasted:
  - host supplies xT [C, N] per batch (one DMA per batch)
  - host supplies w_qkv regrouped as [P, 12, CT, 128]: head-pair column
    groups (q0,k0,q1,k1,...) then v, so each pair's weights arrive in one
    early DMA on the scalar engine's HW-DGE queue (parallel to the x loads
    on the sync queue) and the first scores can start early
  - qT,kT computed channel-major ([ch, n]) with w_qkv as stationary
  - v computed token-major ([n, ch]) with xT chunks as stationary
  - S^T = K @ Q^T per head ([m, n], key-major, two heads row-packed in the
    PE array) so softmax sums arrive via a ones-column in the attn@V matmul
  - attn@V uses masked exp(S^T) full [128,128] chunks as stationary and
    [v | 1] as moving; normalization is one reciprocal + one broadcast
    multiply per query block
  - the head-concat [n, c] block is transposed to channel-major by the DMA
    xbar (dma_start_transpose, ~0.5us per block on the sync queue) instead
    of PE transposes; the xbar's channel interleaving (c = 4p + ct) is
    compensated by reordering w_proj's rows on the host
  - the projection computes y^T (w_proj chunks stationary, outcatT moving)
    so the bias is a per-(partition,chunk) broadcast add fused into the
    PSUM->SBUF move; y is stored transposed and untransposed on the host
Mask is applied as a 0/1 multiply after exp (exp never sees -1e30; scores
are O(10) so no max-subtraction is needed). Block-level structure is
derived from the actual mask argument at call time, so a dense (all-zero)
mask also works.

Scheduling (engines execute their instruction streams in order, so emission
order is the schedule):
  - qkv is computed PAIR-WISE, fused with that pair's score matmuls, so the
    scalar engine's exp chain starts as soon as the first q/k pair exists
    instead of after the whole qkv GEMM.
  - software pipeline one batch ahead: iteration b interleaves batch b's
    attn@V/out with batch b+1's qkv+scores (batch 0 is scored in a
    pair-fused prologue).
  - attnV quarters are emitted BEFORE the score tiles in the t-loop so the
    PE never head-of-line blocks on the psS ring waiting for exp.
  - the projection of query block j is deferred into the next iteration
    (after the next qk pair's matmuls) so the DMA transpose of the
    head-concat never stalls the PE.
  - engine assignment: scalar = exp only, vector = qkv copies +
    normalization + mask (wide groups) + bias add, gpsimd = mask (narrow
    groups) + pad memsets, sync-DGE = x loads + transposes + stores,
    scalar-DGE = weight loads.
"""

import numpy as np
import ml_dtypes

B, N, C = 32, 512, 512
HEADS = 8
HD = C // HEADS
SCALE = HD ** -0.5
NCORES = 8
BPC = B // NCORES  # batches per core
P = 128            # partitions
NT = N // P        # 4 n/m tiles of 128
CT = C // P        # 4 channel tiles of 128

_BF16 = ml_dtypes.bfloat16

_cache = {}


def _mask_structure(mask2d):
    """Derive block structure from the additive mask [n, m]."""
    vis = mask2d == 0.0  # [n, m] True = visible
    assert vis.any(axis=1).all(), "some query attends to nothing"
    # Per key-tile t: storage window [offs, offs+W) is 128-block aligned so
    # every attn@V chunk is a full [128,128] stationary; exp only writes the
    # true visible sub-window [offs+elo, offs+elo+width); the rest of the
    # window that chunks can read ("pads") is memset to zero.
    offs, elos, widths, spans = [], [], [], []
    for t in range(NT):
        sub = vis[:, t * P:(t + 1) * P]  # [n, 128]
        rows = np.nonzero(sub.any(axis=1))[0]
        vlo, vhi = int(rows.min()), int(rows.max()) + 1
        o = (vlo // P) * P
        span = ((vhi + P - 1) // P) * P - o
        offs.append(o)
        elos.append(vlo - o)
        widths.append(vhi - vlo)
        spans.append(span)
    W = max(spans)  # storage pitch (multiple of 128)
    pads = []  # (t, start_col, width) regions read by chunks but not written
    for t in range(NT):
        if elos[t] > 0:
            pads.append((t, 0, elos[t]))
        end = elos[t] + widths[t]
        if end < spans[t]:
            pads.append((t, end, spans[t] - end))
    chunks = []
    for s in range(NT):
        cl = []
        for t in range(NT):
            blk = vis[s * P:(s + 1) * P, t * P:(t + 1) * P]
            if not blk.any():
                continue
            lo, hi = s * P, (s + 1) * P
            assert lo >= offs[t] and hi <= offs[t] + spans[t]
            cl.append((t, lo, hi))
        assert cl, f"query block {s} has no visible key chunks"
        chunks.append(cl)
    return W, offs, elos, widths, pads, chunks


def _uniform_groups(entries):
    """Group (t, start, width) entries into runs with equal width and a
    uniform (t, start) stride, so each run is one strided AP op."""
    groups = []
    by_w = {}
    for e in entries:
        by_w.setdefault(e[2], []).append(e)
    for w, es in sorted(by_w.items()):
        es = sorted(es)
        while es:
            run = [es[0]]
            for e in es[1:]:
                if len(run) == 1:
                    run.append(e)
                else:
                    d_t = run[1][0] - run[0][0]
                    d_s = run[1][1] - run[0][1]
                    if e[0] - run[-1][0] == d_t and e[1] - run[-1][1] == d_s:
                        run.append(e)
            es = [e for e in es if e not in run]
            groups.append((w, run))
    return groups


def _build(W, offs, elos, widths, pads, chunks):
    import concourse.bass as bass
    import concourse.tile as tile
    import concourse.mybir as mybir
    from concourse import bacc
    from concourse.masks import make_identity

    fp32 = mybir.dt.float32
    bf16 = mybir.dt.bfloat16
    AF = mybir.ActivationFunctionType

    nc = bacc.Bacc("TRN2", target_bir_lowering=False, debug=False)

    d_xt = nc.dram_tensor("xt", [BPC, P, CT, N], bf16, kind="ExternalInput")
    d_wqkv = nc.dram_tensor("wqkv", [P, 12, CT, P], bf16, kind="ExternalInput")
    d_wproj = nc.dram_tensor("wproj", [P, CT, C], bf16, kind="ExternalInput")
    d_brep = nc.dram_tensor("brep", [P, C], fp32, kind="ExternalInput")
    d_m01 = nc.dram_tensor("m01", [P, NT, W], bf16, kind="ExternalInput")
    d_y = nc.dram_tensor("y", [BPC, N, C], fp32, kind="ExternalOutput")

    mask_groups = _uniform_groups(
        [(t, elos[t], widths[t]) for t in range(NT)])
    pad_groups = _uniform_groups(pads)
    # widest mask groups go to the (2x faster) vector engine, the rest to
    # gpsimd, splitting the post-exp multiply across both
    msorted = sorted(mask_groups, key=lambda g: -g[0] * len(g[1]))

    with tile.TileContext(nc) as tc:
        with (
            tc.tile_pool(name="singles", bufs=1) as singles,
            tc.tile_pool(name="xt", bufs=3) as xt_pool,
            tc.tile_pool(name="qk", bufs=2) as qk_pool,
            tc.tile_pool(name="vplus", bufs=2) as v_pool,
            tc.tile_pool(name="apair", bufs=9) as a_pool,
            tc.tile_pool(name="oc", bufs=3) as oc_pool,
            tc.tile_pool(name="rec", bufs=4) as rec_pool,
            tc.tile_pool(name="psS", bufs=2, space="PSUM") as psS_pool,
            tc.tile_pool(name="psB", bufs=2, space="PSUM") as psB_pool,
            tc.tile_pool(name="psO", bufs=2, space="PSUM") as psO_pool,
        ):
            # ---- weight loads on the scalar engine's HW-DGE queue (runs in
            # parallel with the x loads on the sync queue) ----
            wq = singles.tile([P, 12, CT, P], bf16)
            for j in range(CT):
                nc.scalar.dma_start(
                    out=wq[:, 2 * j:2 * j + 2],
                    in_=d_wqkv.ap()[:, 2 * j:2 * j + 2])
            nc.scalar.dma_start(out=wq[:, 8:12], in_=d_wqkv.ap()[:, 8:12])

            def xt_load(b, split=False):
                x1 = xt_pool.tile([P, CT, N], bf16, tag="xt")
                if split:
                    # two DMAs so the first qkv matmuls (which only need
                    # the first channel chunks) start ~2us earlier
                    h = CT // 2
                    nc.sync.dma_start(out=x1[:, 0:h], in_=d_xt.ap()[b, :, 0:h])
                    nc.sync.dma_start(out=x1[:, h:], in_=d_xt.ap()[b, :, h:])
                else:
                    nc.sync.dma_start(out=x1, in_=d_xt.ap()[b])
                return x1

            xts = [None] * BPC
            xts[0] = xt_load(0, split=True)

            wproj = singles.tile([P, CT, C], bf16)
            nc.sync.dma_start(out=wproj, in_=d_wproj.ap())
            m01 = singles.tile([P, NT, W], bf16)
            nc.sync.dma_start(out=m01, in_=d_m01.ap())
            brep = singles.tile([P, C], fp32)
            nc.sync.dma_start(out=brep, in_=d_brep.ap())
            ident = singles.tile([P, P], bf16)
            make_identity(nc, ident)
            if BPC > 1:
                xts[1] = xt_load(1)

            def group_ap(base3d, run, w, lead=None):
                """AP over [P, (2,) len(run), w] from a [P, NT, W] view;
                `run` is [(t, start), ...] with uniform stride. With
                lead=(stride, count), adds a leading free dim (head dim)."""
                t0, s0 = run[0][0], run[0][1]
                a = base3d[:, t0, s0:s0 + w]
                step = ((run[1][0] - t0) * W + run[1][1] - s0) \
                    if len(run) > 1 else 1
                dims = [a.ap[0]]
                if lead is not None:
                    dims.append(list(lead))
                dims += [[step, len(run)], [1, w]]
                return bass.AP(tensor=a.tensor, offset=a.offset, ap=dims)

            def new_apair():
                apair = a_pool.tile([P, 2, NT, W], bf16, tag="apair")
                for w, run in pad_groups:
                    nc.gpsimd.memset(
                        group_ap(apair[:, 0], [(t, s) for t, s, _ in run], w,
                                 lead=(NT * W, 2)), 0.0)
                return apair

            def qk_pair(qk, xt, j, copy_eng=None):
                """Channel-major qT (j<CT) and kT for head pair j. The
                pre-block pair's copies go on scalar: the vector queue is
                the straggler at iteration boundaries (norm -> transpose ->
                proj chain) and these two casts were stalling it."""
                for g in range(2):
                    ps = psB_pool.tile([P, N], fp32, tag="psB")
                    for ct in range(CT):
                        nc.tensor.matmul(
                            ps,
                            lhsT=wq[:, 2 * j + g, ct, :],
                            rhs=xt[:, ct, :],
                            start=(ct == 0), stop=(ct == CT - 1))
                    if copy_eng is nc.scalar:
                        nc.scalar.copy(out=qk[:, g * CT + j, :], in_=ps)
                    else:
                        nc.vector.tensor_copy(out=qk[:, g * CT + j, :], in_=ps)

            def v_tile(vplus, xt, t, copy_eng=None):
                """Token-major v for token tile t (all heads). The copy goes
                on scalar in steady state (vector is loaded); the prologue
                passes vector (scalar is saturated by the exp chain there)."""
                ps = psB_pool.tile([P, C], fp32, tag="psB")
                for ct in range(CT):
                    nc.tensor.matmul(
                        ps,
                        lhsT=xt[:, ct, t * P:(t + 1) * P],
                        rhs=wq[:, 8:12, ct, :],
                        start=(ct == 0), stop=(ct == CT - 1))
                eng = nc.scalar if copy_eng is None else copy_eng
                if eng is nc.scalar:
                    nc.scalar.copy(
                        out=vplus[:, t, :, 0:HD],
                        in_=ps.rearrange("p (h d) -> p h d", h=HEADS))
                else:
                    eng.tensor_copy(
                        out=vplus[:, t, :, 0:HD],
                        in_=ps.rearrange("p (h d) -> p h d", h=HEADS))

            def score_tile(qk, apair, j, t):
                """S^T matmuls (row-packed pair) + exp for key tile t."""
                w = widths[t]
                el = elos[t]
                psp = psS_pool.tile([P, 2, N], fp32, tag="psS")
                for hh in range(2):
                    sl = slice(hh * HD, (hh + 1) * HD)
                    nc.tensor.matmul(
                        psp[:, hh, 0:w],
                        lhsT=qk[sl, CT + j, t * P:(t + 1) * P],
                        rhs=qk[sl, j, offs[t] + el:offs[t] + el + w],
                        start=True, stop=True)
                nc.scalar.activation(
                    out=apair[:, :, t, el:el + w], in_=psp[:, :, 0:w],
                    func=AF.Exp)

            def mask_mul(apair, j):
                """0/1 visibility multiply; both heads in one op per group;
                widest groups on vector, the rest on gpsimd."""
                for gi, (w, run) in enumerate(msorted):
                    eng = nc.vector if gi % 2 == 0 else nc.gpsimd
                    r = [(t, s) for t, s, _ in run]
                    eng.tensor_mul(
                        group_ap(apair[:, 0], r, w, lead=(NT * W, 2)),
                        group_ap(apair[:, 0], r, w, lead=(NT * W, 2)),
                        group_ap(m01, r, w, lead=(0, 2)))

            def attnv_quarter(apairs, vplus, oc, s, q, state):
                """Quarter q (0..3) of query block s: two heads' attn @ [v|1]
                matmuls into the current 4-head PSUM bank (start=True only on
                the bank's first matmul), plus the bank's normalization when
                its 4 heads are complete."""
                cl = chunks[s]
                if q % 2 == 0:
                    pso = psO_pool.tile([P, 4, P], fp32, tag="psO")
                    state["pso"] = pso
                pso = state["pso"]
                for hh2 in range(2):
                    hh = (q % 2) * 2 + hh2
                    h = (q // 2) * 4 + hh
                    for ci, (t, lo, hi) in enumerate(cl):
                        nc.tensor.matmul(
                            pso[lo - s * P:hi - s * P, hh, 0:HD + 1],
                            lhsT=apairs[h // 2][
                                :, h % 2, t, lo - offs[t]:hi - offs[t]],
                            rhs=vplus[:, t, h, :],
                            start=(hh == 0 and ci == 0),
                            stop=(hh == 3 and ci == len(cl) - 1),
                            skip_group_check=True)
                if q % 2 == 1:
                    g = q // 2
                    rec = rec_pool.tile([P, 4], fp32, tag="rec")
                    nc.vector.reciprocal(rec, pso[:, :, HD])
                    ra = rec[:, :]
                    rec_b = bass.AP(
                        tensor=ra.tensor, offset=ra.offset,
                        ap=[ra.ap[0], [1, 4], [0, HD]])
                    nc.vector.tensor_mul(
                        oc[:, g * C // 2:(g + 1) * C // 2].rearrange(
                            "p (h d) -> p h d", h=4),
                        pso[:, :, 0:HD], rec_b)

            def transpose_oc(oc, use_pe=False):
                """Channel-major head-concat. Steady state: DMA xbar (frees
                the PE; channel order ct*128+p matches w_proj's host-side
                row order). Drain iteration: PE transposes (the PE is idle
                there and the xbar's ~1.3us would sit on the critical
                norm->transpose->proj->store chain)."""
                ocTs = rec_pool.tile([P, CT, P], bf16, tag="ocTs")
                if not use_pe:
                    nc.sync.dma_start_transpose(out=ocTs, in_=oc)
                else:
                    pst = psB_pool.tile([P, N], bf16, tag="psB")
                    for ct in range(CT):
                        nc.tensor.matmul(
                            pst[:, ct * P:(ct + 1) * P],
                            lhsT=oc[:, ct * P:(ct + 1) * P],
                            rhs=ident, is_transpose=True,
                            start=(ct == 0), stop=(ct == CT - 1),
                            skip_group_check=True)
                    nc.scalar.copy(
                        out=ocTs, in_=pst.rearrange("p (c n) -> p c n", c=CT))
                return ocTs

            def proj_block(b, s, ocTs):
                """Project transposed block, add bias, stream to DRAM."""
                ps = psB_pool.tile([P, C], fp32, tag="psB")
                for ct in range(CT):
                    nc.tensor.matmul(
                        ps,
                        lhsT=ocTs[:, ct, :],
                        rhs=wproj[:, ct, :],
                        start=(ct == 0), stop=(ct == CT - 1))
                ysb = rec_pool.tile([P, C], fp32, tag="ysb")
                nc.vector.tensor_add(ysb, ps, brep)
                nc.sync.dma_start(
                    out=d_y.ap()[b, s * P:(s + 1) * P, :], in_=ysb)

            # ---- prologue: batch 0 qkv + scores, pair-fused ----
            qk_q = [None] * BPC
            vplus_q = [None] * BPC
            apairs_q = [None] * BPC
            qk0 = qk_pool.tile([P, 2 * CT, N], bf16, tag="qk")
            apairs0 = []
            for j in range(CT):
                qk_pair(qk0, xts[0], j)
                apair = new_apair()
                apairs0.append(apair)
                for t in range(NT):
                    score_tile(qk0, apair, j, t)
                mask_mul(apair, j)
            vplus0 = v_pool.tile([P, NT, HEADS, HD + 1], bf16, tag="vplus")
            for t in range(NT):
                v_tile(vplus0, xts[0], t, copy_eng=nc.vector)
            nc.gpsimd.memset(vplus0[:, :, :, HD:HD + 1], 1.0)
            vplus_q[0], apairs_q[0] = vplus0, apairs0

            # ---- main loop: attnV/out of batch b + qkv/scores of b+1 ----
            # The build runs one BLOCK ahead of the attnv consumer: pair 0
            # of batch b+1 is scored before attnv block 0 of batch b, pair
            # j+1 during attnv block j, and block 3 has no build work, so
            # the last pair's exp/mask chain has a full block of slack
            # before iteration b+1's attnv needs it.
            def build_pair(qk_n, apairs_n, xt_n, jp, copy_eng=None):
                qk_pair(qk_n, xt_n, jp, copy_eng=copy_eng)
                apair_n = new_apair()
                apairs_n.append(apair_n)
                return apair_n

            pending = None  # (b, j, ocTs) awaiting projection
            for b in range(BPC):
                build = b + 1 if b + 1 < BPC else None
                if build is not None:
                    if build + 1 < BPC:
                        xts[build + 1] = xt_load(build + 1)
                    qk_n = qk_pool.tile([P, 2 * CT, N], bf16, tag="qk")
                    vplus_n = v_pool.tile(
                        [P, NT, HEADS, HD + 1], bf16, tag="vplus")
                    apairs_n = []
                    ap0 = build_pair(qk_n, apairs_n, xts[build], 0,
                                     copy_eng=nc.scalar)
                    for t in range(NT):
                        score_tile(qk_n, ap0, 0, t)
                    mask_mul(ap0, 0)
                for j in range(CT):
                    bp = j + 1 if build is not None and j + 1 < CT else None
                    if bp is not None:
                        qk_pair(qk_n, xts[build], bp)
                    if pending is not None:
                        proj_block(*pending)
                        pending = None
                    if bp is not None:
                        apair_n = new_apair()
                        apairs_n.append(apair_n)
                    oc = oc_pool.tile([P, C], bf16, tag="oc")
                    st = {}
                    for t in range(NT):
                        attnv_quarter(apairs_q[b], vplus_q[b], oc, j, t, st)
                        if bp is not None:
                            score_tile(qk_n, apair_n, bp, t)
                    if bp is not None:
                        mask_mul(apair_n, bp)
                    if build is not None:
                        v_tile(vplus_n, xts[build], j, copy_eng=nc.scalar)
                    pending = (b, j, transpose_oc(oc, use_pe=build is None))
                if build is not None:
                    nc.gpsimd.memset(vplus_n[:, :, :, HD:HD + 1], 1.0)
                    vplus_q[build] = vplus_n
                    apairs_q[build] = apairs_n
            proj_block(*pending)

    nc.compile()
    return nc


def _prep(x, w_qkv, w_proj, b_proj, mask):
    x = np.asarray(x, np.float32)
    w_qkv = np.asarray(w_qkv, np.float32)
    w_proj = np.asarray(w_proj, np.float32)
    b_proj = np.asarray(b_proj, np.float32)
    mask2d = np.asarray(mask, np.float32).reshape(N, N)

    W, offs, elos, widths, pads, chunks = _mask_structure(mask2d)

    ws = w_qkv.copy()
    ws[:, :C] *= SCALE  # fold q scaling into the weights
    # regroup to [P, 12, CT, 128]: g=2j -> q pair j, g=2j+1 -> k pair j,
    # g=8+t -> v chunk t; each [:, g0:g1] slice is one contiguous DMA
    w3 = ws.reshape(CT, P, 3, CT, P).transpose(1, 2, 3, 0, 4)
    wqkv_b = np.empty((P, 12, CT, P), _BF16)
    wqkv_b[:, 0:8] = w3[:, 0:2].transpose(0, 2, 1, 3, 4).reshape(
        P, 8, CT, P).astype(_BF16)
    wqkv_b[:, 8:12] = w3[:, 2].astype(_BF16)

    # w_proj rows in the DMA-xbar channel order c = ct*128 + p
    wproj_b = np.ascontiguousarray(
        w_proj.reshape(CT, P, C).transpose(1, 0, 2)).astype(_BF16)
    brep = np.tile(b_proj.reshape(1, C), (P, 1)).astype(np.float32)

    vis = (mask2d == 0.0)
    m01 = np.zeros((P, NT, W), np.float32)
    for t in range(NT):
        # m01[p, t, c] = visible(query=offs[t]+c, key=t*128+p)
        hi = min(offs[t] + W, N)
        m01[:, t, 0:hi - offs[t]] = vis[offs[t]:hi, t * P:(t + 1) * P].T
    m01_b = m01.astype(_BF16)

    # xT per core: [NCORES, BPC, P, CT, N] (x[b, n, ct*128+p] -> [b,p,ct,n])
    xt = np.ascontiguousarray(
        x.reshape(NCORES, BPC, N, CT, P).transpose(0, 1, 4, 3, 2)
    ).astype(_BF16)
    key = (W, tuple(offs), tuple(elos), tuple(widths),
           tuple(pads), tuple(tuple(c) for c in chunks))
    return xt, wqkv_b, wproj_b, brep, m01_b, key


LAST_RESULTS = None


def kernel(x, w_qkv, w_proj, b_proj, mask, _trace=False):
    global LAST_RESULTS
    from concourse import bass_utils

    xt, wqkv_b, wproj_b, brep, m01_b, key = _prep(
        x, w_qkv, w_proj, b_proj, mask)
    W, offs, elos, widths, pads, chunks = key

    if key not in _cache:
        _cache[key] = _build(W, list(offs), list(elos), list(widths),
                             list(pads), [list(c) for c in chunks])
    nc = _cache[key]

    in_maps = []
    for core in range(NCORES):
        in_maps.append({
            "xt": xt[core],
            "wqkv": wqkv_b,
            "wproj": wproj_b,
            "brep": brep,
            "m01": m01_b,
        })
    res = bass_utils.run_bass_kernel_spmd(
        nc, in_maps, core_ids=list(range(NCORES)), trace=_trace)
    LAST_RESULTS = res
    y = np.concatenate([res.results[c]["y"] for c in range(NCORES)], axis=0)
    return y.reshape(B, N, C).astype(np.float32)


# revision 27
# speedup vs baseline: 1.0074x; 1.0074x over previous
"""Trainium2 Bass kernel for batched 8-head local-window attention.

Shapes (hardcoded): x [32, 512, 512], w_qkv [512, 1536], w_proj [512, 512],
b_proj [512], mask [1, 1, 512, 512] additive (0 or -1e30).

Strategy: data-parallel over batch across 8 cores (4 batch elements each).
All matmuls in bf16 (fp32 PSUM accumulation). Layouts chosen so that no
on-device data movement is wasted:
  - host supplies xT [C, N] per batch (one DMA per batch)
  - host supplies w_qkv regrouped as [P, 12, CT, 128]: head-pair column
    groups (q0,k0,q1,k1,...) then v, so each pair's weights arrive in one
    early DMA on the scalar engine's HW-DGE queue (parallel to the x loads
    on the sync queue) and the first scores can start early
  - qT,kT computed channel-major ([ch, n]) with w_qkv as stationary
  - v computed token-major ([n, ch]) with xT chunks as stationary
  - S^T = K @ Q^T per head ([m, n], key-major, two heads row-packed in the
    PE array) so softmax sums arrive via a ones-column in the attn@V matmul
  - attn@V uses masked exp(S^T) full [128,128] chunks as stationary and
    [v | 1] as moving; normalization is one reciprocal + one broadcast
    multiply per query block
  - the head-concat [n, c] block is transposed to channel-major by the DMA
    xbar (dma_start_transpose, ~0.5us per block on the sync queue) instead
    of PE transposes; the xbar's channel interleaving (c = 4p + ct) is
    compensated by reordering w_proj's rows on the host
  - the projection computes y^T (w_proj chunks stationary, outcatT moving)
    so the bias is a per-(partition,chunk) broadcast add fused into the
    PSUM->SBUF move; y is stored transposed and untransposed on the host
Mask is applied as a 0/1 multiply after exp (exp never sees -1e30; scores
are O(10) so no max-subtraction is needed). Block-level structure is
derived from the actual mask argument at call time, so a dense (all-zero)
mask also works.

Scheduling (engines execute their instruction streams in order, so emission
order is the schedule):
  - qkv is computed PAIR-WISE, fused with that pair's score matmuls, so the
    scalar engine's exp chain starts as soon as the first q/k pair exists
    instead of after the whole qkv GEMM.
  - software pipeline one batch ahead: iteration b interleaves batch b's
    attn@V/out with batch b+1's qkv+scores (batch 0 is scored in a
    pair-fused prologue).
  - attnV quarters are emitted BEFORE the score tiles in the t-loop so the
    PE never head-of-line blocks on the psS ring waiting for exp.
  - the projection of query block j is deferred into the next iteration
    (after the next qk pair's matmuls) so the DMA transpose of the
    head-concat never stalls the PE.
  - engine assignment: scalar = exp only, vector = qkv copies +
    normalization + mask (wide groups) + bias add, gpsimd = mask (narrow
    groups) + pad memsets, sync-DGE = x loads + transposes + stores,
    scalar-DGE = weight loads.
"""

import numpy as np
import ml_dtypes

B, N, C = 32, 512, 512
HEADS = 8
HD = C // HEADS
SCALE = HD ** -0.5
NCORES = 8
BPC = B // NCORES  # batches per core
P = 128            # partitions
NT = N // P        # 4 n/m tiles of 128
CT = C // P        # 4 channel tiles of 128

_BF16 = ml_dtypes.bfloat16

_cache = {}


def _mask_structure(mask2d):
    """Derive block structure from the additive mask [n, m]."""
    vis = mask2d == 0.0  # [n, m] True = visible
    assert vis.any(axis=1).all(), "some query attends to nothing"
    # Per key-tile t: storage window [offs, offs+W) is 128-block aligned so
    # every attn@V chunk is a full [128,128] stationary; exp only writes the
    # true visible sub-window [offs+elo, offs+elo+width); the rest of the
    # window that chunks can read ("pads") is memset to zero.
    offs, elos, widths, spans = [], [], [], []
    for t in range(NT):
        sub = vis[:, t * P:(t + 1) * P]  # [n, 128]
        rows = np.nonzero(sub.any(axis=1))[0]
        vlo, vhi = int(rows.min()), int(rows.max()) + 1
        o = (vlo // P) * P
        span = ((vhi + P - 1) // P) * P - o
        offs.append(o)
        elos.append(vlo - o)
        widths.append(vhi - vlo)
        spans.append(span)
    W = max(spans)  # storage pitch (multiple of 128)
    pads = []  # (t, start_col, width) regions read by chunks but not written
    for t in range(NT):
        if elos[t] > 0:
            pads.append((t, 0, elos[t]))
        end = elos[t] + widths[t]
        if end < spans[t]:
            pads.append((t, end, spans[t] - end))
    chunks = []
    for s in range(NT):
        cl = []
        for t in range(NT):
            blk = vis[s * P:(s + 1) * P, t * P:(t + 1) * P]
            if not blk.any():
                continue
            lo, hi = s * P, (s + 1) * P
            assert lo >= offs[t] and hi <= offs[t] + spans[t]
            cl.append((t, lo, hi))
        assert cl, f"query block {s} has no visible key chunks"
        chunks.append(cl)
    return W, offs, elos, widths, pads, chunks


def _uniform_groups(entries):
    """Group (t, start, width) entries into runs with equal width and a
    uniform (t, start) stride, so each run is one strided AP op."""
    groups = []
    by_w = {}
    for e in entries:
        by_w.setdefault(e[2], []).append(e)
    for w, es in sorted(by_w.items()):
        es = sorted(es)
        while es:
            run = [es[0]]
            for e in es[1:]:
                if len(run) == 1:
                    run.append(e)
                else:
                    d_t = run[1][0] - run[0][0]
                    d_s = run[1][1] - run[0][1]
                    if e[0] - run[-1][0] == d_t and e[1] - run[-1][1] == d_s:
                        run.append(e)
            es = [e for e in es if e not in run]
            groups.append((w, run))
    return groups


def _build(W, offs, elos, widths, pads, chunks):
    import concourse.bass as bass
    import concourse.tile as tile
    import concourse.mybir as mybir
    from concourse import bacc
    from concourse.masks import make_identity

    fp32 = mybir.dt.float32
    bf16 = mybir.dt.bfloat16
    AF = mybir.ActivationFunctionType

    nc = bacc.Bacc("TRN2", target_bir_lowering=False, debug=False)

    d_xt = nc.dram_tensor("xt", [BPC, P, CT, N], bf16, kind="ExternalInput")
    d_wqkv = nc.dram_tensor("wqkv", [P, 12, CT, P], bf16, kind="ExternalInput")
    d_wproj = nc.dram_tensor("wproj", [P, CT, C], bf16, kind="ExternalInput")
    d_brep = nc.dram_tensor("brep", [P, C], fp32, kind="ExternalInput")
    d_m01 = nc.dram_tensor("m01", [P, NT, W], bf16, kind="ExternalInput")
    d_y = nc.dram_tensor("y", [BPC, N, C], fp32, kind="ExternalOutput")

    mask_groups = _uniform_groups(
        [(t, elos[t], widths[t]) for t in range(NT)])
    pad_groups = _uniform_groups(pads)
    # widest mask groups go to the (2x faster) vector engine, the rest to
    # gpsimd, splitting the post-exp multiply across both
    msorted = sorted(mask_groups, key=lambda g: -g[0] * len(g[1]))

    with tile.TileContext(nc) as tc:
        with (
            tc.tile_pool(name="singles", bufs=1) as singles,
            tc.tile_pool(name="xt", bufs=3) as xt_pool,
            tc.tile_pool(name="qk", bufs=2) as qk_pool,
            tc.tile_pool(name="vplus", bufs=2) as v_pool,
            tc.tile_pool(name="apair", bufs=9) as a_pool,
            tc.tile_pool(name="oc", bufs=3) as oc_pool,
            tc.tile_pool(name="rec", bufs=4) as rec_pool,
            tc.tile_pool(name="psS", bufs=2, space="PSUM") as psS_pool,
            tc.tile_pool(name="psB", bufs=2, space="PSUM") as psB_pool,
            tc.tile_pool(name="psO", bufs=2, space="PSUM") as psO_pool,
        ):
            # ---- weight loads on the scalar engine's HW-DGE queue (runs in
            # parallel with the x loads on the sync queue) ----
            wq = singles.tile([P, 12, CT, P], bf16)
            for j in range(CT):
                nc.scalar.dma_start(
                    out=wq[:, 2 * j:2 * j + 2],
                    in_=d_wqkv.ap()[:, 2 * j:2 * j + 2])
            nc.scalar.dma_start(out=wq[:, 8:12], in_=d_wqkv.ap()[:, 8:12])

            def xt_load(b, split=False):
                x1 = xt_pool.tile([P, CT, N], bf16, tag="xt")
                if split:
                    # two DMAs so the first qkv matmuls (which only need
                    # the first channel chunks) start ~2us earlier
                    h = CT // 2
                    nc.sync.dma_start(out=x1[:, 0:h], in_=d_xt.ap()[b, :, 0:h])
                    nc.sync.dma_start(out=x1[:, h:], in_=d_xt.ap()[b, :, h:])
                else:
                    nc.sync.dma_start(out=x1, in_=d_xt.ap()[b])
                return x1

            xts = [None] * BPC
            xts[0] = xt_load(0)

            wproj = singles.tile([P, CT, C], bf16)
            nc.sync.dma_start(out=wproj, in_=d_wproj.ap())
            m01 = singles.tile([P, NT, W], bf16)
            nc.sync.dma_start(out=m01, in_=d_m01.ap())
            brep = singles.tile([P, C], fp32)
            nc.sync.dma_start(out=brep, in_=d_brep.ap())
            ident = singles.tile([P, P], bf16)
            make_identity(nc, ident)
            if BPC > 1:
                xts[1] = xt_load(1)

            def group_ap(base3d, run, w, lead=None):
                """AP over [P, (2,) len(run), w] from a [P, NT, W] view;
                `run` is [(t, start), ...] with uniform stride. With
                lead=(stride, count), adds a leading free dim (head dim)."""
                t0, s0 = run[0][0], run[0][1]
                a = base3d[:, t0, s0:s0 + w]
                step = ((run[1][0] - t0) * W + run[1][1] - s0) \
                    if len(run) > 1 else 1
                dims = [a.ap[0]]
                if lead is not None:
                    dims.append(list(lead))
                dims += [[step, len(run)], [1, w]]
                return bass.AP(tensor=a.tensor, offset=a.offset, ap=dims)

            def new_apair():
                apair = a_pool.tile([P, 2, NT, W], bf16, tag="apair")
                for w, run in pad_groups:
                    nc.gpsimd.memset(
                        group_ap(apair[:, 0], [(t, s) for t, s, _ in run], w,
                                 lead=(NT * W, 2)), 0.0)
                return apair

            def qk_pair(qk, xt, j, copy_eng=None):
                """Channel-major qT (j<CT) and kT for head pair j. The
                pre-block pair's copies go on scalar: the vector queue is
                the straggler at iteration boundaries (norm -> transpose ->
                proj chain) and these two casts were stalling it."""
                for g in range(2):
                    ps = psB_pool.tile([P, N], fp32, tag="psB")
                    for ct in range(CT):
                        nc.tensor.matmul(
                            ps,
                            lhsT=wq[:, 2 * j + g, ct, :],
                            rhs=xt[:, ct, :],
                            start=(ct == 0), stop=(ct == CT - 1))
                    if copy_eng is nc.scalar:
                        nc.scalar.copy(out=qk[:, g * CT + j, :], in_=ps)
                    else:
                        nc.vector.tensor_copy(out=qk[:, g * CT + j, :], in_=ps)

            def v_tile(vplus, xt, t, copy_eng=None):
                """Token-major v for token tile t (all heads). The copy goes
                on scalar in steady state (vector is loaded); the prologue
                passes vector (scalar is saturated by the exp chain there)."""
                ps = psB_pool.tile([P, C], fp32, tag="psB")
                for ct in range(CT):
                    nc.tensor.matmul(
                        ps,
                        lhsT=xt[:, ct, t * P:(t + 1) * P],
                        rhs=wq[:, 8:12, ct, :],
                        start=(ct == 0), stop=(ct == CT - 1))
                eng = nc.scalar if copy_eng is None else copy_eng
                if eng is nc.scalar:
                    nc.scalar.copy(
                        out=vplus[:, t, :, 0:HD],
                        in_=ps.rearrange("p (h d) -> p h d", h=HEADS))
                else:
                    eng.tensor_copy(
                        out=vplus[:, t, :, 0:HD],
                        in_=ps.rearrange("p (h d) -> p h d", h=HEADS))

            def score_tile(qk, apair, j, t):
                """S^T matmuls (row-packed pair) + exp for key tile t."""
                w = widths[t]
                el = elos[t]
                psp = psS_pool.tile([P, 2, N], fp32, tag="psS")
                for hh in range(2):
                    sl = slice(hh * HD, (hh + 1) * HD)
                    nc.tensor.matmul(
                        psp[:, hh, 0:w],
                        lhsT=qk[sl, CT + j, t * P:(t + 1) * P],
                        rhs=qk[sl, j, offs[t] + el:offs[t] + el + w],
                        start=True, stop=True)
                nc.scalar.activation(
                    out=apair[:, :, t, el:el + w], in_=psp[:, :, 0:w],
                    func=AF.Exp)

            def mask_mul(apair, j):
                """0/1 visibility multiply; both heads in one op per group;
                widest groups on vector, the rest on gpsimd."""
                for gi, (w, run) in enumerate(msorted):
                    eng = nc.vector if gi % 2 == 0 else nc.gpsimd
                    r = [(t, s) for t, s, _ in run]
                    eng.tensor_mul(
                        group_ap(apair[:, 0], r, w, lead=(NT * W, 2)),
                        group_ap(apair[:, 0], r, w, lead=(NT * W, 2)),
                        group_ap(m01, r, w, lead=(0, 2)))

            def attnv_quarter(apairs, vplus, oc, s, q, state):
                """Quarter q (0..3) of query block s: two heads' attn @ [v|1]
                matmuls into the current 4-head PSUM bank (start=True only on
                the bank's first matmul), plus the bank's normalization when
                its 4 heads are complete."""
                cl = chunks[s]
                if q % 2 == 0:
                    pso = psO_pool.tile([P, 4, P], fp32, tag="psO")
                    state["pso"] = pso
                pso = state["pso"]
                for hh2 in range(2):
                    hh = (q % 2) * 2 + hh2
                    h = (q // 2) * 4 + hh
                    for ci, (t, lo, hi) in enumerate(cl):
                        nc.tensor.matmul(
                            pso[lo - s * P:hi - s * P, hh, 0:HD + 1],
                            lhsT=apairs[h // 2][
                                :, h % 2, t, lo - offs[t]:hi - offs[t]],
                            rhs=vplus[:, t, h, :],
                            start=(hh == 0 and ci == 0),
                            stop=(hh == 3 and ci == len(cl) - 1),
                            skip_group_check=True)
                if q % 2 == 1:
                    g = q // 2
                    rec = rec_pool.tile([P, 4], fp32, tag="rec")
                    nc.vector.reciprocal(rec, pso[:, :, HD])
                    ra = rec[:, :]
                    rec_b = bass.AP(
                        tensor=ra.tensor, offset=ra.offset,
                        ap=[ra.ap[0], [1, 4], [0, HD]])
                    nc.vector.tensor_mul(
                        oc[:, g * C // 2:(g + 1) * C // 2].rearrange(
                            "p (h d) -> p h d", h=4),
                        pso[:, :, 0:HD], rec_b)

            def transpose_oc(oc, use_pe=False):
                """Channel-major head-concat. Steady state: DMA xbar (frees
                the PE; channel order ct*128+p matches w_proj's host-side
                row order). Drain iteration: PE transposes (the PE is idle
                there and the xbar's ~1.3us would sit on the critical
                norm->transpose->proj->store chain)."""
                ocTs = rec_pool.tile([P, CT, P], bf16, tag="ocTs")
                if not use_pe:
                    nc.sync.dma_start_transpose(out=ocTs, in_=oc)
                else:
                    # per-half pipelining: chunks 0-1 depend only on the
                    # first norm half, and their copy releases proj's first
                    # stationary without waiting for the second half
                    pst = psB_pool.tile([P, N], bf16, tag="psB")
                    h = CT // 2
                    for half in range(2):
                        for ct in range(half * h, (half + 1) * h):
                            nc.tensor.matmul(
                                pst[:, ct * P:(ct + 1) * P],
                                lhsT=oc[:, ct * P:(ct + 1) * P],
                                rhs=ident, is_transpose=True,
                                start=(ct == half * h),
                                stop=(ct == (half + 1) * h - 1),
                                skip_group_check=True)
                        nc.scalar.copy(
                            out=ocTs[:, half * h:(half + 1) * h],
                            in_=pst[:, half * h * P:(half + 1) * h * P]
                            .rearrange("p (c n) -> p c n", c=h))
                return ocTs

            def proj_block(b, s, ocTs, split=False):
                """Project transposed block, add bias, stream to DRAM. The
                final call splits by output half so the last ysb+store
                overlaps the second half's matmuls."""
                ps = psB_pool.tile([P, C], fp32, tag="psB")
                ysb = rec_pool.tile([P, C], fp32, tag="ysb")
                halves = 2 if split else 1
                hc = C // halves
                for ha in range(halves):
                    sl = slice(ha * hc, (ha + 1) * hc)
                    for ct in range(CT):
                        nc.tensor.matmul(
                            ps[:, sl],
                            lhsT=ocTs[:, ct, :],
                            rhs=wproj[:, ct, sl],
                            start=(ct == 0), stop=(ct == CT - 1),
                            skip_group_check=True)
                    nc.vector.tensor_add(
                        ysb[:, sl], ps[:, sl], brep[:, sl])
                    nc.sync.dma_start(
                        out=d_y.ap()[b, s * P:(s + 1) * P, sl],
                        in_=ysb[:, sl])

            # ---- prologue: batch 0 qkv + scores, pair-fused ----
            qk_q = [None] * BPC
            vplus_q = [None] * BPC
            apairs_q = [None] * BPC
            qk0 = qk_pool.tile([P, 2 * CT, N], bf16, tag="qk")
            apairs0 = []
            for j in range(CT):
                qk_pair(qk0, xts[0], j)
                apair = new_apair()
                apairs0.append(apair)
                for t in range(NT):
                    score_tile(qk0, apair, j, t)
                mask_mul(apair, j)
            vplus0 = v_pool.tile([P, NT, HEADS, HD + 1], bf16, tag="vplus")
            for t in range(NT):
                v_tile(vplus0, xts[0], t, copy_eng=nc.vector)
            nc.gpsimd.memset(vplus0[:, :, :, HD:HD + 1], 1.0)
            vplus_q[0], apairs_q[0] = vplus0, apairs0

            # ---- main loop: attnV/out of batch b + qkv/scores of b+1 ----
            # The build runs one BLOCK ahead of the attnv consumer: pair 0
            # of batch b+1 is scored before attnv block 0 of batch b, pair
            # j+1 during attnv block j, and block 3 has no build work, so
            # the last pair's exp/mask chain has a full block of slack
            # before iteration b+1's attnv needs it.
            def build_pair(qk_n, apairs_n, xt_n, jp, copy_eng=None):
                qk_pair(qk_n, xt_n, jp, copy_eng=copy_eng)
                apair_n = new_apair()
                apairs_n.append(apair_n)
                return apair_n

            pending = None  # (b, j, ocTs) awaiting projection
            for b in range(BPC):
                build = b + 1 if b + 1 < BPC else None
                if build is not None:
                    if build + 1 < BPC:
                        xts[build + 1] = xt_load(build + 1)
                    qk_n = qk_pool.tile([P, 2 * CT, N], bf16, tag="qk")
                    vplus_n = v_pool.tile(
                        [P, NT, HEADS, HD + 1], bf16, tag="vplus")
                    apairs_n = []
                    ap0 = build_pair(qk_n, apairs_n, xts[build], 0,
                                     copy_eng=nc.scalar)
                    for t in range(NT):
                        score_tile(qk_n, ap0, 0, t)
                    mask_mul(ap0, 0)
                for j in range(CT):
                    bp = j + 1 if build is not None and j + 1 < CT else None
                    if bp is not None:
                        qk_pair(qk_n, xts[build], bp)
                    if pending is not None:
                        proj_block(*pending)
                        pending = None
                    if bp is not None:
                        apair_n = new_apair()
                        apairs_n.append(apair_n)
                    oc = oc_pool.tile([P, C], bf16, tag="oc")
                    st = {}
                    for t in range(NT):
                        attnv_quarter(apairs_q[b], vplus_q[b], oc, j, t, st)
                        if bp is not None:
                            score_tile(qk_n, apair_n, bp, t)
                    if bp is not None:
                        mask_mul(apair_n, bp)
                    if build is not None:
                        v_tile(vplus_n, xts[build], j, copy_eng=nc.scalar)
                    pending = (b, j, transpose_oc(oc, use_pe=build is None))
                if build is not None:
                    nc.gpsimd.memset(vplus_n[:, :, :, HD:HD + 1], 1.0)
                    vplus_q[build] = vplus_n
                    apairs_q[build] = apairs_n
            proj_block(*pending, split=True)

    nc.compile()
    return nc


def _prep(x, w_qkv, w_proj, b_proj, mask):
    x = np.asarray(x, np.float32)
    w_qkv = np.asarray(w_qkv, np.float32)
    w_proj = np.asarray(w_proj, np.float32)
    b_proj = np.asarray(b_proj, np.float32)
    mask2d = np.asarray(mask, np.float32).reshape(N, N)

    W, offs, elos, widths, pads, chunks = _mask_structure(mask2d)

    ws = w_qkv.copy()
    ws[:, :C] *= SCALE  # fold q scaling into the weights
    # regroup to [P, 12, CT, 128]: g=2j -> q pair j, g=2j+1 -> k pair j,
    # g=8+t -> v chunk t; each [:, g0:g1] slice is one contiguous DMA
    w3 = ws.reshape(CT, P, 3, CT, P).transpose(1, 2, 3, 0, 4)
    wqkv_b = np.empty((P, 12, CT, P), _BF16)
    wqkv_b[:, 0:8] = w3[:, 0:2].transpose(0, 2, 1, 3, 4).reshape(
        P, 8, CT, P).astype(_BF16)
    wqkv_b[:, 8:12] = w3[:, 2].astype(_BF16)

    # w_proj rows in the DMA-xbar channel order c = ct*128 + p
    wproj_b = np.ascontiguousarray(
        w_proj.reshape(CT, P, C).transpose(1, 0, 2)).astype(_BF16)
    brep = np.tile(b_proj.reshape(1, C), (P, 1)).astype(np.float32)

    vis = (mask2d == 0.0)
    m01 = np.zeros((P, NT, W), np.float32)
    for t in range(NT):
        # m01[p, t, c] = visible(query=offs[t]+c, key=t*128+p)
        hi = min(offs[t] + W, N)
        m01[:, t, 0:hi - offs[t]] = vis[offs[t]:hi, t * P:(t + 1) * P].T
    m01_b = m01.astype(_BF16)

    # xT per core: [NCORES, BPC, P, CT, N] (x[b, n, ct*128+p] -> [b,p,ct,n])
    xt = np.ascontiguousarray(
        x.reshape(NCORES, BPC, N, CT, P).transpose(0, 1, 4, 3, 2)
    ).astype(_BF16)
    key = (W, tuple(offs), tuple(elos), tuple(widths),
           tuple(pads), tuple(tuple(c) for c in chunks))
    return xt, wqkv_b, wproj_b, brep, m01_b, key


LAST_RESULTS = None


def kernel(x, w_qkv, w_proj, b_proj, mask, _trace=False):
    global LAST_RESULTS
    from concourse import bass_utils

    xt, wqkv_b, wproj_b, brep, m01_b, key = _prep(
        x, w_qkv, w_proj, b_proj, mask)
    W, offs, elos, widths, pads, chunks = key

    if key not in _cache:
        _cache[key] = _build(W, list(offs), list(elos), list(widths),
                             list(pads), [list(c) for c in chunks])
    nc = _cache[key]

    in_maps = []
    for core in range(NCORES):
        in_maps.append({
            "xt": xt[core],
            "wqkv": wqkv_b,
            "wproj": wproj_b,
            "brep": brep,
            "m01": m01_b,
        })
    res = bass_utils.run_bass_kernel_spmd(
        nc, in_maps, core_ids=list(range(NCORES)), trace=_trace)
    LAST_RESULTS = res
    y = np.concatenate([res.results[c]["y"] for c in range(NCORES)], axis=0)
    return y.reshape(B, N, C).astype(np.float32)
